# revision 1
# baseline (speedup 1.0000x reference)
"""DeBERTa-style 12-layer transformer on 8 TRN2 NeuronCores.

Sharding: data-parallel over batch (B=8 -> 1 sequence per core, no
collectives). Weights are host-prepped (transposed/tiled/bf16) and
replicated per core. Relative-position tables are expanded on host into
per-layer T1/T2 tables; the (q,k)-dependent gather is done on device via
matmul + a strided "skew" DMA read from a DRAM scratch buffer.
"""

import math
import numpy as np
import ml_dtypes

import concourse.bacc as bacc
import concourse.bass as bass
import concourse.mybir as mybir
from concourse import tile
from concourse.bass_utils import run_bass_kernel_spmd
from concourse.masks import make_identity

BF = ml_dtypes.bfloat16
F16 = np.float16
bf16 = mybir.dt.bfloat16
fp16 = mybir.dt.float16
f32 = mybir.dt.float32

V = 32768; H = 768; NH = 12; D = 64; L = 12; FI = 2048
S = 512; B = 8; BK = 32; MAXP = 512; EPS = 1e-7
SCALE = 1.0 / math.sqrt(3 * D)
NQT = S // 128      # 4 token tiles
NHT = H // 128      # 6 hidden tiles
WEXP = 640          # C-block width (per-tile expansion window)
MASK_NEG = -60000.0


# ---------------------------------------------------------------- host math
def _beta_delta():
    """bucket(delta)+31 for delta in [-511, 511], indexed by delta+511."""
    delta = np.arange(-(S - 1), S)
    sign = np.sign(delta)
    mid = BK // 2
    abs_pos = np.where((delta < mid) & (delta > -mid), mid - 1,
                       np.minimum(np.abs(delta), MAXP - 1))
    log_pos = np.ceil(np.log(abs_pos / mid) / math.log((MAXP - 1) / mid)
                      * (mid - 1)).astype(np.int64) + mid
    bucket = np.where(abs_pos <= mid, delta, log_pos * sign).astype(np.int64)
    return bucket + BK - 1


def _ln_np(x):
    m = x.mean(-1, keepdims=True)
    v = x.var(-1, keepdims=True)
    return (x - m) / np.sqrt(v + EPS)


# ---------------------------------------------------------------- builder
def _build(n_layers):
    nc = bacc.Bacc("TRN2", target_bir_lowering=False, num_devices=B)

    # ---- dram inputs (host-prepped layouts) ----
    wqk = nc.dram_tensor("wqk", [n_layers, NHT, 12, 128, 128], fp16, kind="ExternalInput")
    wvg = nc.dram_tensor("wvg", [n_layers, NHT, 3, 128, 512], fp16, kind="ExternalInput")
    wout = nc.dram_tensor("wout", [n_layers, NHT, 2, 128, 512], fp16, kind="ExternalInput")
    w1 = nc.dram_tensor("w1", [n_layers, NHT, 8, 128, 512], fp16, kind="ExternalInput")
    w2 = nc.dram_tensor("w2", [n_layers, 16, 2, 128, 512], fp16, kind="ExternalInput")
    t1d = nc.dram_tensor("t1d", [n_layers, NH // 2, 128, 1024], fp16, kind="ExternalInput")
    t2d = nc.dram_tensor("t2d", [n_layers, NH // 2, 128, 1024], fp16, kind="ExternalInput")
    bqkd = nc.dram_tensor("bqkd", [n_layers, 128, 12], f32, kind="ExternalInput")
    bvgd = nc.dram_tensor("bvgd", [n_layers, 1, 1536], fp16, kind="ExternalInput")
    boutd = nc.dram_tensor("boutd", [n_layers, 1, 1024], fp16, kind="ExternalInput")
    x0d = nc.dram_tensor("x0d", [NQT, 128, H], f32, kind="ExternalInput")
    maskd = nc.dram_tensor("maskd", [128, NQT], f32, kind="ExternalInput")
    yd = nc.dram_tensor("yd", [NQT, 128, H], f32, kind="ExternalOutput")

    # dram scratch for positional C blocks (per layer, reused)
    c1d = nc.dram_tensor("c1d", [2, NH, NQT, 128, WEXP], fp16, kind="Internal")
    c2d = nc.dram_tensor("c2d", [2, NH, NQT, 128, WEXP], fp16, kind="Internal")
    CBLK = 128 * WEXP

    with tile.TileContext(nc) as tc:
        import contextlib
        ctx = contextlib.ExitStack()
        with ctx:
            pp = ctx.enter_context(tc.tile_pool(name="persist", bufs=1))
            sb = ctx.enter_context(tc.tile_pool(name="work", bufs=2))
            sb3 = ctx.enter_context(tc.tile_pool(name="work3", bufs=3))
            wpool = ctx.enter_context(tc.tile_pool(name="wts", bufs=4))
            ps_mm = ctx.enter_context(tc.tile_pool(name="psmm", bufs=3, space="PSUM"))
            ps_aux = ctx.enter_context(tc.tile_pool(name="psaux", bufs=3, space="PSUM"))
            ps_ctx = ctx.enter_context(tc.tile_pool(name="psctx", bufs=2, space="PSUM"))

            # persistent tiles
            x = [pp.tile([128, H], f32, name=f"x{qt}") for qt in range(NQT)]
            ident = pp.tile([128, 128], fp16, name="ident")
            make_identity(nc, ident[:])
            ones_col = pp.tile([128, 1], bf16, name="ones_col")
            nc.gpsimd.memset(ones_col[:], 1.0)
            ones_row = pp.tile([1, 128], fp16, name="ones_row")
            nc.gpsimd.memset(ones_row[:], 1.0)
            one_f32 = pp.tile([1, 1], f32, name="one_f32")
            nc.gpsimd.memset(one_f32[:], 1.0)
            maskb = pp.tile([128, NQT], f32, name="maskb")
            nc.sync.dma_start(maskb[:], maskd[:])
            for qt in range(NQT):
                nc.sync.dma_start(x[qt][:], x0d[qt, :, :])

            # ---------------- helpers ----------------
            def layer_norm(chunks, out_writer):
                """chunks: list of APs [128, w<=512] covering the row.
                out_writer(rstd_ap, negb_ap): emits normalize ops."""
                nst = len(chunks)
                stats = sb.tile([128, nst * 6], f32, tag="lnstats")
                for i, cap in enumerate(chunks):
                    nc.vector.bn_stats(stats[:, i * 6:(i + 1) * 6], cap)
                mv = sb.tile([128, 2], f32, tag="lnmv")
                nc.vector.bn_aggr(mv[:], stats[:])
                veps = sb.tile([128, 1], f32, tag="lnveps")
                nc.vector.tensor_scalar_add(veps[:], mv[:, 1:2], EPS)
                iv = sb.tile([128, 1], f32, tag="lniv")
                nc.vector.reciprocal(iv[:], veps[:])
                rstd = sb.tile([128, 1], f32, tag="lnrstd")
                nc.scalar.sqrt(rstd[:], iv[:])
                negb = sb.tile([128, 1], f32, tag="lnnegb")
                nc.vector.scalar_tensor_tensor(
                    negb[:], mv[:, 0:1], -1.0, rstd[:],
                    op0=mybir.AluOpType.mult, op1=mybir.AluOpType.mult)
                out_writer(rstd[:], negb[:])

            def ln_one(t, width, tag, bufs=1):
                if width == H:
                    chunks = [t[:, 0:384], t[:, 384:768]]
                else:
                    chunks = [t[:, c * 512:(c + 1) * 512] for c in range(width // 512)]
                o = sb.tile([128, width], fp16, tag=tag, name=tag, bufs=bufs)
                def wr(rstd, negb, t=t, o=o):
                    nc.scalar.activation(o[:], t[:],
                                         mybir.ActivationFunctionType.Identity,
                                         bias=negb, scale=rstd)
                layer_norm(chunks, wr)
                return o

            def ln_to_bf16(src_tiles, width, tagp):
                return [ln_one(src_tiles[qt], width, f"{tagp}{qt}")
                        for qt in range(NQT)]

            def transpose_h(tiles_bf16, nh_tiles, tag):
                """[128,q tiles][*, nh_tiles*128 wide] -> nh_tiles x [128, 512] (hT layout)."""
                outs = []
                for hc in range(nh_tiles):
                    pt = ps_aux.tile([128, 512], fp16, tag="aux")
                    for qt in range(NQT):
                        nc.tensor.transpose(pt[:, qt * 128:(qt + 1) * 128],
                                            tiles_bf16[qt][:, hc * 128:(hc + 1) * 128],
                                            ident[:])
                    o = sb.tile([128, 512], fp16, tag=f"{tag}{hc}", name=f"{tag}{hc}", bufs=1)
                    nc.scalar.copy(o[:], pt[:])
                    outs.append(o)
                return outs

            # ---------------- layers ----------------
            for li in range(n_layers):
                par = li % 2
                # ---- attention input LN + transpose ----
                hs = ln_to_bf16(x, H, "hs")                       # 4 x [128,768] bf16
                hsT = transpose_h(hs, NHT, "hsT")           # 6 x [128,512] bf16

                # ---- QK^T projection: 12 o-tiles [128, 512] (o on partitions) ----
                qkT = []
                bqk_sb = sb.tile([128, 12], f32, tag="bqk")
                nc.sync.dma_start(bqk_sb[:], bqkd[li, :, :])
                for ot in range(12):
                    po = ps_mm.tile([128, 512], f32, tag="mm")
                    for hc in range(NHT):
                        wt = wpool.tile([128, 128], fp16, tag="wqk")
                        nc.sync.dma_start(wt[:], wqk[li, hc, ot, :, :])
                        nc.tensor.matmul(po[:], wt[:], hsT[hc][:],
                                         start=(hc == 0), stop=(hc == NHT - 1))
                    o = sb.tile([128, 512], fp16, tag=f"qkT{ot}", name=f"qkT{ot}", bufs=1)
                    sc = SCALE if ot < 6 else 1.0
                    nc.scalar.activation(o[:], po[:],
                                         mybir.ActivationFunctionType.Identity,
                                         bias=bqk_sb[:, ot:ot + 1], scale=sc)
                    qkT.append(o)

                # ---- VG projection: natural layout [tok, o] ----
                v_sb = [sb.tile([128, H], bf16, tag=f"v{tt}", name=f"v{tt}", bufs=1) for tt in range(NQT)]
                g_sb = [sb.tile([128, H], fp16, tag=f"g{tt}", name=f"g{tt}", bufs=1) for tt in range(NQT)]
                for tt in range(NQT):
                    for oc in range(3):
                        po = ps_mm.tile([128, 512], f32, tag="mm")
                        for hc in range(NHT):
                            wt = wpool.tile([128, 512], fp16, tag="wvg")
                            nc.sync.dma_start(wt[:], wvg[li, hc, oc, :, :])
                            nc.tensor.matmul(po[:], hsT[hc][:, tt * 128:(tt + 1) * 128],
                                             wt[:], start=(hc == 0), stop=False)
                        bv = wpool.tile([1, 512], fp16, tag="bvg", bufs=2)
                        nc.sync.dma_start(bv[:], bvgd[li, :, oc * 512:(oc + 1) * 512])
                        nc.tensor.matmul(po[:], ones_row[:], bv[:],
                                         start=False, stop=True)
                        # split columns into v / g
                        lo = oc * 512
                        if lo + 512 <= H:
                            nc.scalar.copy(v_sb[tt][:, lo:lo + 512], po[:])
                        elif lo >= H:
                            nc.scalar.copy(g_sb[tt][:, lo - H:lo - H + 512], po[:])
                        else:
                            cut = H - lo
                            nc.scalar.copy(v_sb[tt][:, lo:H], po[:, :cut])
                            nc.scalar.copy(g_sb[tt][:, 0:512 - cut], po[:, cut:])

                # ---- attention per head ----
                ctx_sb = [sb.tile([128, H], fp16, tag=f"ctx{qt}", name=f"ctx{qt}", bufs=1) for qt in range(NQT)]
                t1_sb = t2_sb = None
                for h in range(NH):
                    hp = (h % 2) * 64
                    qT_h = qkT[h // 2][hp:hp + 64, :]
                    kT_h = qkT[6 + h // 2][hp:hp + 64, :]
                    if h % 2 == 0:
                        t1_sb = sb3.tile([128, 1024], fp16, tag="t1", bufs=2)
                        nc.sync.dma_start(t1_sb[:], t1d[li, h // 2, :, :])
                        t2_sb = sb3.tile([128, 1024], fp16, tag="t2", bufs=2)
                        nc.sync.dma_start(t2_sb[:], t2d[li, h // 2, :, :])
                    # C blocks -> DRAM
                    for tsb, lhs_full, cdram in (
                            (t1_sb, qT_h, c1d), (t2_sb, kT_h, c2d)):
                        for bt in range(NQT):
                            j0 = 384 - 128 * bt
                            pa = ps_mm.tile([128, 512], f32, tag="mm")
                            nc.tensor.matmul(pa[:], lhs_full[:, bt * 128:(bt + 1) * 128],
                                             tsb[hp:hp + 64, j0:j0 + 512], start=True, stop=True)
                            pb = ps_aux.tile([128, 128], f32, tag="aux")
                            nc.tensor.matmul(pb[:], lhs_full[:, bt * 128:(bt + 1) * 128],
                                             tsb[hp:hp + 64, j0 + 512:j0 + 640], start=True, stop=True)
                            stg = sb3.tile([128, WEXP], fp16, tag="cstg")
                            nc.vector.tensor_copy(stg[:, 0:512], pa[:])
                            nc.vector.tensor_copy(stg[:, 512:WEXP], pb[:])
                            nc.sync.dma_start(cdram[par, h, bt, :, :], stg[:])
                    # skew reads
                    c1base = ((par * NH + h) * NQT) * CBLK
                    c2base = c1base
                    c2p_sb = []
                    for qt in range(NQT):
                        t = sb3.tile([128, 512], fp16, tag=f"c2p{qt}", name=f"c2p{qt}", bufs=2)
                        ap = bass.AP(c1d, c1base + qt * CBLK + 127, [[WEXP - 1, 128], [1, 512]])
                        nc.sync.dma_start(t[:], ap)
                        c2p_sb.append(t)
                    p2c_sb = []
                    for kt in range(NQT):
                        t = sb3.tile([128, 512], fp16, tag="p2c", name="p2c")
                        ap = bass.AP(c2d, c2base + kt * CBLK + 127, [[WEXP - 1, 128], [1, 512]])
                        nc.sync.dma_start(t[:], ap)
                        p2c_sb.append(t)
                    # scores / softmax / ctx
                    ctxT_ps = ps_ctx.tile([64, 512], f32, tag="ctxT")
                    den_ps = ps_aux.tile([1, 512], f32, tag="aux")
                    pT_tiles = []
                    for kt in range(NQT):
                        ps_s = ps_mm.tile([128, 512], f32, tag="mm")
                        nc.tensor.matmul(ps_s[:], kT_h[:, kt * 128:(kt + 1) * 128],
                                         qT_h[:], start=True, stop=True)
                        pc2 = ps_aux.tile([128, 512], fp16, tag="aux")
                        for qt in range(NQT):
                            nc.tensor.transpose(pc2[:, qt * 128:(qt + 1) * 128],
                                                c2p_sb[qt][:, kt * 128:(kt + 1) * 128],
                                                ident[:])
                        c2pT = sb3.tile([128, 512], fp16, tag="c2pT")
                        nc.scalar.copy(c2pT[:], pc2[:])
                        rel = sb3.tile([128, 512], fp16, tag="rel")
                        nc.vector.tensor_add(rel[:], c2pT[:], p2c_sb[kt][:])
                        s_sb = sb3.tile([128, 512], f32, tag="s", bufs=2)
                        nc.vector.scalar_tensor_tensor(
                            s_sb[:], ps_s[:], maskb[:, kt:kt + 1], rel[:],
                            op0=mybir.AluOpType.add, op1=mybir.AluOpType.add)
                        pT = sb3.tile([128, 512], bf16, tag="pT", name="pT")
                        nc.scalar.activation(pT[:], s_sb[:],
                                             mybir.ActivationFunctionType.Exp)
                        pT_tiles.append(pT)
                        nc.tensor.matmul(den_ps[:], ones_col[:], pT[:],
                                         start=(kt == 0), stop=(kt == NQT - 1),
                                         skip_group_check=True)
                        nc.tensor.matmul(ctxT_ps[:], v_sb[kt][:, h * 64:(h + 1) * 64],
                                         pT[:], start=(kt == 0), stop=(kt == NQT - 1),
                                         skip_group_check=True)
                    # denom -> per-q-tile scale vectors
                    rec = sb.tile([1, 512], f32, tag="rec")
                    nc.vector.reciprocal(rec[:], den_ps[:])
                    rs_ps = ps_aux.tile([128, NQT], f32, tag="aux")
                    for qt in range(NQT):
                        nc.tensor.transpose(rs_ps[:, qt:qt + 1],
                                            rec[:, qt * 128:(qt + 1) * 128],
                                            one_f32[:])
                    rs_sb = sb.tile([128, NQT], f32, tag="rs")
                    nc.vector.tensor_copy(rs_sb[:], rs_ps[:])
                    # ctx^T -> ctx with scaling
                    ctxT_sb = sb.tile([64, 512], fp16, tag="ctxTsb")
                    nc.scalar.copy(ctxT_sb[:], ctxT_ps[:])
                    for qt in range(NQT):
                        pc = ps_aux.tile([128, 64], fp16, tag="aux")
                        nc.tensor.transpose(pc[:], ctxT_sb[:, qt * 128:(qt + 1) * 128],
                                            ident[:64, :64])
                        nc.scalar.activation(ctx_sb[qt][:, h * 64:(h + 1) * 64], pc[:],
                                             mybir.ActivationFunctionType.Copy,
                                             scale=rs_sb[:, qt:qt + 1])

                # ---- gate + LN + out proj ----
                cgn = []
                for qt in range(NQT):
                    gg = sb.tile([128, H], fp16, tag="gg")
                    nc.scalar.activation(gg[:], g_sb[qt][:],
                                         mybir.ActivationFunctionType.Gelu)
                    t = sb.tile([128, H], f32, tag="cg", name="cg")
                    nc.vector.tensor_mul(t[:], ctx_sb[qt][:], gg[:])
                    cgn.append(ln_one(t, H, f"cgn{qt}"))
                cgT = transpose_h(cgn, NHT, "cgT")
                bo = wpool.tile([1, 1024], fp16, tag="bout", bufs=1)
                nc.sync.dma_start(bo[:], boutd[li, :, :])
                for qt in range(NQT):
                    for oc in range(2):
                        po = ps_mm.tile([128, 512], f32, tag="mm")
                        for hc in range(NHT):
                            wt = wpool.tile([128, 512], fp16, tag="wout")
                            nc.sync.dma_start(wt[:], wout[li, hc, oc, :, :])
                            nc.tensor.matmul(po[:], cgT[hc][:, qt * 128:(qt + 1) * 128],
                                             wt[:], start=(hc == 0), stop=False)
                        nc.tensor.matmul(po[:], ones_row[:],
                                         bo[:, oc * 512:(oc + 1) * 512],
                                         start=False, stop=True)
                        w = 512 if oc == 0 else H - 512
                        nc.vector.tensor_add(x[qt][:, oc * 512:oc * 512 + w],
                                             x[qt][:, oc * 512:oc * 512 + w],
                                             po[:, :w])

                # ---- FFN ----
                h2 = ln_to_bf16(x, H, "h2")
                h2T = transpose_h(h2, NHT, "h2T")
                un = []
                for qt in range(NQT):
                    u = sb.tile([128, FI], f32, tag="u", name="u")
                    for oc in range(8):
                        po = ps_mm.tile([128, 512], f32, tag="mm")
                        for hc in range(NHT):
                            wt = wpool.tile([128, 512], fp16, tag="w1")
                            nc.sync.dma_start(wt[:], w1[li, hc, oc, :, :])
                            nc.tensor.matmul(po[:], h2T[hc][:, qt * 128:(qt + 1) * 128],
                                             wt[:], start=(hc == 0), stop=(hc == NHT - 1))
                        if oc < 4:
                            nc.scalar.copy(u[:, oc * 512:(oc + 1) * 512], po[:])
                        else:
                            gt = sb.tile([128, 512], f32, tag="ffng")
                            nc.scalar.activation(gt[:], po[:],
                                                 mybir.ActivationFunctionType.Gelu_apprx_tanh)
                            lo = (oc - 4) * 512
                            nc.vector.tensor_mul(u[:, lo:lo + 512],
                                                 u[:, lo:lo + 512], gt[:])
                    un.append(ln_one(u, FI, f"un{qt}"))
                unT = []
                for ic in range(16):
                    pt = ps_aux.tile([128, 512], fp16, tag="aux")
                    for qt in range(NQT):
                        nc.tensor.transpose(pt[:, qt * 128:(qt + 1) * 128],
                                            un[qt][:, ic * 128:(ic + 1) * 128],
                                            ident[:])
                    o = sb.tile([128, 512], fp16, tag=f"unT{ic}", name=f"unT{ic}", bufs=1)
                    nc.scalar.copy(o[:], pt[:])
                    unT.append(o)
                for qt in range(NQT):
                    for oc in range(2):
                        po = ps_mm.tile([128, 512], f32, tag="mm")
                        for ic in range(16):
                            wt = wpool.tile([128, 512], fp16, tag="w2")
                            nc.sync.dma_start(wt[:], w2[li, ic, oc, :, :])
                            nc.tensor.matmul(po[:], unT[ic][:, qt * 128:(qt + 1) * 128],
                                             wt[:], start=(ic == 0), stop=(ic == 15))
                        w = 512 if oc == 0 else H - 512
                        nc.vector.tensor_add(x[qt][:, oc * 512:oc * 512 + w],
                                             x[qt][:, oc * 512:oc * 512 + w],
                                             po[:, :w])

            # ---- output ----
            for qt in range(NQT):
                nc.sync.dma_start(yd[qt, :, :], x[qt][:])

    nc.finalize()
    return nc


_CACHE = {}


def _get_nc(n_layers):
    if n_layers not in _CACHE:
        _CACHE[n_layers] = _build(n_layers)
    return _CACHE[n_layers]


# ---------------------------------------------------------------- host prep
def _prep_shared(word_emb, rel_emb, rel_g, rel_b, Wqk, bqk, Wvg, bvg, Wout,
                 bout, W1, W2, n_layers):
    beta = _beta_delta()                     # [1023]
    idx_c2p = beta[1022 - np.arange(1023)]   # T1: delta = 511 - j
    idx_p2c = beta[np.arange(1023)]          # T2: delta = j - 511
    rel = _ln_np(rel_emb.astype(np.float64)).astype(np.float32) * rel_g + rel_b

    d = {}
    t1 = np.zeros((n_layers, NH, 64, 1024), np.float32)  # packed to pairs below
    t2 = np.zeros((n_layers, NH, 64, 1024), np.float32)
    wqk_t = np.zeros((n_layers, NHT, 12, 128, 128), np.float32)
    wvg_t = np.zeros((n_layers, NHT, 3, 128, 512), np.float32)
    wout_t = np.zeros((n_layers, NHT, 2, 128, 512), np.float32)
    w1_t = np.zeros((n_layers, NHT, 8, 128, 512), np.float32)
    w2_t = np.zeros((n_layers, 16, 2, 128, 512), np.float32)
    bqk_t = np.zeros((n_layers, 128, 12), np.float32)
    bvg_t = np.zeros((n_layers, 1, 1536), np.float32)
    bout_t = np.zeros((n_layers, 1, 1024), np.float32)
    for li in range(n_layers):
        pos = rel @ Wqk[li].T + bqk[li]          # [63, 1536]
        qpos = pos[:, :H].reshape(63, NH, 64)
        kpos = pos[:, H:].reshape(63, NH, 64)
        # T1[j] = kpos[beta(511-j)], T2[j] = qpos[beta(j-511)] * SCALE
        t1[li, :, :, :1023] = kpos[idx_c2p].transpose(1, 2, 0)
        t2[li, :, :, :1023] = qpos[idx_p2c].transpose(1, 2, 0) * SCALE

        wqkT = Wqk[li].T.copy()                  # [768, 1536]
        wqk_t[li] = wqkT.reshape(NHT, 128, 12, 128).swapaxes(1, 2)
        wvg_t[li] = Wvg[li].T.reshape(NHT, 128, 3, 512).swapaxes(1, 2)
        woutT = np.zeros((H, 1024), np.float32)
        woutT[:, :H] = Wout[li].T
        wout_t[li] = woutT.reshape(NHT, 128, 2, 512).swapaxes(1, 2)
        w1_t[li] = W1[li].T.reshape(NHT, 128, 8, 512).swapaxes(1, 2)
        w2T = np.zeros((FI, 1024), np.float32)
        w2T[:, :H] = W2[li].T
        w2_t[li] = w2T.reshape(16, 128, 2, 512).swapaxes(1, 2)
        bqk_t[li] = bqk[li].reshape(12, 128).T
        bvg_t[li, 0] = bvg[li]
        bout_t[li, 0, :H] = bout[li]

    d["wqk"] = wqk_t.astype(F16)
    d["wvg"] = wvg_t.astype(F16)
    d["wout"] = wout_t.astype(F16)
    d["w1"] = w1_t.astype(F16)
    d["w2"] = w2_t.astype(F16)
    d["t1d"] = t1.reshape(n_layers, NH // 2, 128, 1024).astype(F16)
    d["t2d"] = t2.reshape(n_layers, NH // 2, 128, 1024).astype(F16)
    d["bqkd"] = bqk_t
    d["bvgd"] = bvg_t.astype(F16)
    d["boutd"] = bout_t.astype(F16)
    return d


def _make_in_maps(inputs, n_layers):
    input_ids = np.asarray(inputs["input_ids"])
    attention_mask = np.asarray(inputs["attention_mask"])
    word_emb = np.asarray(inputs["word_emb"], np.float32)

    shared = _prep_shared(
        word_emb, np.asarray(inputs["rel_emb"], np.float32),
        np.asarray(inputs["rel_g"], np.float32), np.asarray(inputs["rel_b"], np.float32),
        np.asarray(inputs["Wqk"], np.float32), np.asarray(inputs["bqk"], np.float32),
        np.asarray(inputs["Wvg"], np.float32), np.asarray(inputs["bvg"], np.float32),
        np.asarray(inputs["Wout"], np.float32), np.asarray(inputs["bout"], np.float32),
        np.asarray(inputs["W1"], np.float32), np.asarray(inputs["W2"], np.float32),
        n_layers)

    in_maps = []
    for b in range(B):
        m = dict(shared)
        x0 = _ln_np(word_emb[input_ids[:, b]].astype(np.float64)).astype(np.float32)
        m["x0d"] = x0.reshape(NQT, 128, H)
        mb = np.where(attention_mask[b, 0, 0, :], MASK_NEG, 0.0).astype(np.float32)
        m["maskd"] = mb.reshape(NQT, 128).T.copy()
        in_maps.append(m)
    return in_maps


def run(inputs, n_layers=L, trace=False):
    nc = _get_nc(n_layers)
    in_maps = _make_in_maps(inputs, n_layers)
    res = run_bass_kernel_spmd(nc, in_maps, core_ids=list(range(B)), trace=trace)
    out = np.zeros((S, B, H), np.float32)
    for b in range(B):
        out[:, b, :] = res.results[b]["yd"].reshape(S, H)
    return out, res


def kernel(**inputs) -> np.ndarray:
    out, _ = run(inputs, L)
    return out


# ------------------------------------------------------- timing-only runner
def make_timed_runner(n_layers, in_maps):
    """Build a persistent jitted PJRT callable over 8 cores for wall-clock
    timing (the axon NTFF profile hook is unavailable in this container)."""
    import jax
    from jax.sharding import Mesh, PartitionSpec, NamedSharding
    from jax.experimental.shard_map import shard_map
    from concourse import bass2jax

    nc = _get_nc(n_layers)
    bass2jax.install_neuronx_cc_hook()
    partition_name = nc.partition_id_tensor.name if nc.partition_id_tensor else None
    in_names, out_names, out_avals, zero_outs = [], [], [], []
    import concourse.mybir as _mb
    for alloc in nc.m.functions[0].allocations:
        if not isinstance(alloc, _mb.MemoryLocationSet):
            continue
        name = alloc.memorylocations[0].name
        if alloc.kind == "ExternalInput":
            if name != partition_name:
                in_names.append(name)
        elif alloc.kind == "ExternalOutput":
            out_names.append(name)
            shape = tuple(alloc.tensor_shape)
            dtype = _mb.dt.np(alloc.dtype)
            out_avals.append(jax.core.ShapedArray(shape, dtype))
            zero_outs.append(np.zeros(shape, dtype))
    n_params = len(in_names)
    n_outs = len(out_avals)
    all_in_names = list(in_names) + out_names
    if partition_name is not None:
        all_in_names = all_in_names + [partition_name]

    def _body(*args):
        operands = list(args)
        if partition_name is not None:
            operands.append(bass2jax.partition_id_tensor())
        outs = bass2jax._bass_exec_p.bind(
            *operands, out_avals=tuple(out_avals), in_names=tuple(all_in_names),
            out_names=tuple(out_names), lowering_input_output_aliases=(),
            sim_require_finite=True, sim_require_nnan=True, nc=nc)
        return tuple(outs)

    n_cores = B
    devices = jax.devices()[:n_cores]
    mesh = Mesh(np.asarray(devices), ("core",))
    P = PartitionSpec
    sharded = jax.jit(
        shard_map(_body, mesh=mesh, in_specs=(P("core"),) * (n_params + n_outs),
                  out_specs=(P("core"),) * n_outs, check_rep=False),
        keep_unused=True)

    concat_in = [
        np.concatenate([np.asarray(in_maps[c][nm]) for c in range(n_cores)], axis=0)
        for nm in in_names]
    concat_zeros = [np.zeros((n_cores * z.shape[0], *z.shape[1:]), z.dtype)
                    for z in zero_outs]
    shard = NamedSharding(mesh, P("core"))
    dev_in = [jax.device_put(a, shard) for a in concat_in]
    dev_zeros = [jax.device_put(a, shard) for a in concat_zeros]

    def call():
        outs = sharded(*dev_in, *dev_zeros)
        jax.block_until_ready(outs)
        return outs

    return call

